# revision 11
# baseline (speedup 1.0000x reference)
"""Trainium2 Bass kernel for nn_AttentionGenerator (gnn_message_passing).

Reference math:
    f = einsum('oc,bctv->botv', Wf, feat) + bf          # 1x1 conv, Cout=64
    s_i = einsum('c,bctv->btv', Wa[:64], f)
    s_j = einsum('c,bctv->btv', Wa[64:], f)
    score[b,t,i,j] = s_i[b,t,i] + s_j[b,t,j] + ba
    atten = (exp(leaky_relu(score)) * A) / row_sum

Because f only enters through the two dot products, fold Wf/bf/Wa/ba on
the host into u1 = w1@Wf, u2 = w2@Wf (length-256 vectors) and the scalar
c0 = (w1+w2)@bf + ba.  The device then computes, per (b,t,v), two
channel contractions (TensorEngine), an 18x18 broadcast-add + LeakyReLU
+ exp*A + row-normalize (Vector/Scalar engines).  Memory bound: reads
151 MB of feat, writes 10.6 MB.

Sharding: pure data parallel — batch B=32 split across 8 NeuronCores
(4 batches each), tiny params replicated, no cross-core comms.
"""

import json
import numpy as np
from contextlib import ExitStack

B, Cin, T, V = 32, 256, 256, 18
NCORES = 8
BPC = B // NCORES  # batches per core
TV = T * V
PB = 128  # t-block size (partition dim)
NTB = T // PB

_cached_nc = None


def _legalize_waits_json(bir_json):
    """Split instructions carrying >1 sync wait into single-wait NoOps plus
    the original instruction.  The walrus build in this container accepts at
    most ONE sync-wait command per instruction struct; concourse's Tile
    scheduler freely attaches several.  Hoisting the extra waits onto NoOps
    immediately before the instruction (same engine stream, same position)
    preserves semantics exactly — engines execute their stream in order."""
    bir = json.loads(bir_json)
    ctr = 0
    for fn in bir.get("functions", []):
        for blk in fn.get("blocks", []):
            insts = blk.get("instructions")
            if not insts:
                continue
            out = []
            for inst in insts:
                si = inst.get("sync_info") or {}
                waits = si.get("on_wait") or []
                if len(waits) > 1:
                    for w in waits[:-1]:
                        out.append(
                            {
                                "engine": inst.get("engine"),
                                "ins": [],
                                "name": f"wsplit-{ctr}",
                                "opcode": "NoOp",
                                "outs": [],
                                "sync_info": {"on_update": [], "on_wait": [w]},
                            }
                        )
                        ctr += 1
                    si = dict(si)
                    si["on_wait"] = [waits[-1]]
                    inst = dict(inst)
                    inst["sync_info"] = si
                out.append(inst)
            blk["instructions"] = out
    return json.dumps(bir).encode()


_wait_patch_done = False


def _install_wait_legalizer():
    global _wait_patch_done
    if _wait_patch_done:
        return
    import concourse.bass_utils as bass_utils
    import concourse.bass2jax as bass2jax

    orig = bass_utils.compile_bir_kernel

    def wrapped(bir_json, tmpdir, neff_name="file.neff"):
        return orig(_legalize_waits_json(bir_json), tmpdir, neff_name)

    bass_utils.compile_bir_kernel = wrapped
    bass2jax.compile_bir_kernel = wrapped
    _wait_patch_done = True


def _build_nc():
    import concourse.bass as bass
    import concourse.mybir as mybir
    import concourse.tile as tile
    from concourse.alu_op_type import AluOpType

    f32 = mybir.dt.float32
    nc = bass.Bass()
    feat = nc.dram_tensor("feat", [BPC, Cin, T, V], f32, kind="ExternalInput")
    # wmat[k, c, o]: o-th contraction vector (u1/u2), c-chunk k of 128
    wmat = nc.dram_tensor("wmat", [2, 128, 2], f32, kind="ExternalInput")
    amat = nc.dram_tensor("amat", [V, V], f32, kind="ExternalInput")
    cmat = nc.dram_tensor("cmat", [1, 1], f32, kind="ExternalInput")
    out = nc.dram_tensor("out", [BPC, T, V, V], f32, kind="ExternalOutput")

    with ExitStack() as ctx:
        tc = ctx.enter_context(tile.TileContext(nc))
        singles = ctx.enter_context(tc.tile_pool(name="singles", bufs=1))
        fpool = ctx.enter_context(tc.tile_pool(name="fpool", bufs=3))
        pspool = ctx.enter_context(tc.tile_pool(name="pspool", bufs=2, space="PSUM"))
        spool = ctx.enter_context(tc.tile_pool(name="spool", bufs=2))
        work = ctx.enter_context(tc.tile_pool(name="work", bufs=4))
        opool = ctx.enter_context(tc.tile_pool(name="opool", bufs=4))

        w_t = singles.tile([128, 2, 2], f32)
        nc.sync.dma_start(out=w_t, in_=wmat[:, :, :].rearrange("k p o -> p k o"))
        a_bc = singles.tile([128, V, V], f32)
        nc.sync.dma_start(out=a_bc, in_=amat[:, :].partition_broadcast(128))
        c0_t = singles.tile([128, 1], f32)
        nc.sync.dma_start(out=c0_t, in_=cmat[0, :].partition_broadcast(128))

        # Absorb the const-DMA waits on cheap ops so steady-state instructions
        # carry fewer sync waits (less NoOp splitting at compile).
        warm_ps = pspool.tile([2, 2], f32, tag="warm")
        nc.tensor.matmul(
            out=warm_ps, lhsT=w_t[:, 0, :], rhs=w_t[:, 0, :], start=True, stop=True
        )
        scratch_c = singles.tile([128, 1], f32)
        nc.vector.tensor_copy(out=scratch_c, in_=c0_t)
        scratch_a = singles.tile([128, V, V], f32)
        nc.vector.tensor_copy(out=scratch_a, in_=a_bc)

        for b in range(BPC):
            # feat[b] as [c_within_chunk(partitions), chunk, t*v]
            f_t = fpool.tile([128, 2, TV], f32)
            nc.sync.dma_start(
                out=f_t,
                in_=feat[b, :, :, :].rearrange("(k p) t v -> p k (t v)", p=128),
            )
            # s[o, t*v] = sum_c u_o[c] * feat[c, t*v]: w chunk stationary
            # (2-column LDW), feat moving in 512-wide slices, accumulated
            # over the two c-chunks in PSUM, staged to s_sb in SBUF.
            s_sb = spool.tile([2, TV], f32)
            NPS = TV // 1536  # 3 psum tiles of [2, 1536] (3 banks each)
            for sl in range(NPS):
                ps = pspool.tile([2, 1536], f32, tag="ps")
                for s in range(3):
                    lo = s * 512
                    for k in range(2):
                        nc.tensor.matmul(
                            out=ps[:, lo : lo + 512],
                            lhsT=w_t[:, k, :],
                            rhs=f_t[:, k, sl * 1536 + lo : sl * 1536 + lo + 512],
                            start=(k == 0),
                            stop=(k == 1),
                        )
                nc.scalar.copy(out=s_sb[:, sl * 1536 : (sl + 1) * 1536], in_=ps)
            for tb in range(NTB):
                # SBUF->SBUF scatter: s_sb[o, (t v)] row -> [t, v] tile
                s1t = work.tile([128, V], f32)
                nc.gpsimd.dma_start(
                    out=s1t,
                    in_=s_sb[0:1, tb * PB * V : (tb + 1) * PB * V].rearrange(
                        "o (t v) -> o t v", v=V
                    ),
                )
                s2t = work.tile([128, V], f32)
                nc.gpsimd.dma_start(
                    out=s2t,
                    in_=s_sb[1:2, tb * PB * V : (tb + 1) * PB * V].rearrange(
                        "o (t v) -> o t v", v=V
                    ),
                )
                sc = work.tile([128, V, V], f32)
                s1b = bass.AP(
                    tensor=s1t.tensor,
                    offset=s1t.offset,
                    ap=[s1t.ap[0], [1, V], [0, V]],
                )
                s2b = bass.AP(
                    tensor=s2t.tensor,
                    offset=s2t.offset,
                    ap=[s2t.ap[0], [0, V], [1, V]],
                )
                # sc = (s1 + c0) + s2
                nc.vector.scalar_tensor_tensor(
                    out=sc,
                    in0=s1b,
                    scalar=c0_t[:, :],
                    in1=s2b,
                    op0=AluOpType.add,
                    op1=AluOpType.add,
                )
                # LeakyReLU(x) = max(x, 0.1*x)
                lr = work.tile([128, V, V], f32)
                nc.vector.scalar_tensor_tensor(
                    out=lr,
                    in0=sc,
                    scalar=0.1,
                    in1=sc,
                    op0=AluOpType.mult,
                    op1=AluOpType.max,
                )
                ex = work.tile([128, V, V], f32)
                nc.scalar.activation(
                    out=ex, in_=lr, func=mybir.ActivationFunctionType.Exp
                )
                exa = work.tile([128, V, V], f32)
                nc.vector.tensor_mul(out=exa, in0=ex, in1=a_bc)
                ssum = work.tile([128, V], f32)
                nc.vector.reduce_sum(out=ssum, in_=exa, axis=mybir.AxisListType.X)
                rec = work.tile([128, V], f32)
                nc.vector.reciprocal(out=rec, in_=ssum)
                att = opool.tile([128, V, V], f32)
                rbc = bass.AP(
                    tensor=rec.tensor,
                    offset=rec.offset,
                    ap=[rec.ap[0], [1, V], [0, V]],
                )
                nc.vector.tensor_mul(out=att, in0=exa, in1=rbc)
                nc.gpsimd.dma_start(out=out[b, tb * PB : (tb + 1) * PB], in_=att)
    return nc


def _prep_params(Wf, bf, Wa, ba):
    w1, w2 = Wa[:64].astype(np.float64), Wa[64:].astype(np.float64)
    Wf64, bf64 = Wf.astype(np.float64), bf.astype(np.float64)
    u1 = w1 @ Wf64
    u2 = w2 @ Wf64
    c0 = float(w1 @ bf64 + w2 @ bf64 + float(ba[0]))
    wmat = np.stack([u1, u2], axis=-1).reshape(2, 128, 2).astype(np.float32)
    cmat = np.full((1, 1), c0, dtype=np.float32)
    return wmat, cmat


def get_nc():
    global _cached_nc
    if _cached_nc is None:
        _cached_nc = _build_nc()
    return _cached_nc


def kernel(feat, A, Wf, bf, Wa, ba):
    _install_wait_legalizer()
    from concourse.bass_utils import run_bass_kernel_spmd

    feat = np.ascontiguousarray(np.asarray(feat, dtype=np.float32))
    A = np.ascontiguousarray(np.asarray(A, dtype=np.float32))
    wmat, cmat = _prep_params(
        np.asarray(Wf, np.float32),
        np.asarray(bf, np.float32),
        np.asarray(Wa, np.float32),
        np.asarray(ba, np.float32),
    )

    nc = get_nc()
    in_maps = [
        {
            "feat": feat[i * BPC : (i + 1) * BPC],
            "wmat": wmat,
            "amat": A,
            "cmat": cmat,
        }
        for i in range(NCORES)
    ]
    res = run_bass_kernel_spmd(nc, in_maps, core_ids=list(range(NCORES)))
    return np.concatenate([r["out"] for r in res.results], axis=0)


# revision 12
# speedup vs baseline: 1.1125x; 1.1125x over previous
"""Trainium2 Bass kernel for nn_AttentionGenerator (gnn_message_passing).

Reference math:
    f = einsum('oc,bctv->botv', Wf, feat) + bf          # 1x1 conv, Cout=64
    s_i = einsum('c,bctv->btv', Wa[:64], f)
    s_j = einsum('c,bctv->btv', Wa[64:], f)
    score[b,t,i,j] = s_i[b,t,i] + s_j[b,t,j] + ba
    atten = (exp(leaky_relu(score)) * A) / row_sum

Because f only enters through the two dot products, fold Wf/bf/Wa/ba on
the host into u1 = w1@Wf, u2 = w2@Wf (length-256 vectors) and the scalar
c0 = (w1+w2)@bf + ba.  The device then computes, per (b,t,v), two
channel contractions (TensorEngine), an 18x18 broadcast-add + LeakyReLU
+ exp*A + row-normalize (Vector/Scalar engines).  Memory bound: reads
151 MB of feat, writes 10.6 MB.

Sharding: pure data parallel — batch B=32 split across 8 NeuronCores
(4 batches each), tiny params replicated, no cross-core comms.
"""

import json
import numpy as np
from contextlib import ExitStack

B, Cin, T, V = 32, 256, 256, 18
NCORES = 8
BPC = B // NCORES  # batches per core
TV = T * V
PB = 128  # t-block size (partition dim)
NTB = T // PB

_cached_nc = None


def _legalize_waits_json(bir_json):
    """Split instructions carrying >1 sync wait into single-wait NoOps plus
    the original instruction.  The walrus build in this container accepts at
    most ONE sync-wait command per instruction struct; concourse's Tile
    scheduler freely attaches several.  Hoisting the extra waits onto NoOps
    immediately before the instruction (same engine stream, same position)
    preserves semantics exactly — engines execute their stream in order."""
    bir = json.loads(bir_json)
    ctr = 0
    for fn in bir.get("functions", []):
        for blk in fn.get("blocks", []):
            insts = blk.get("instructions")
            if not insts:
                continue
            out = []
            for inst in insts:
                si = inst.get("sync_info") or {}
                waits = si.get("on_wait") or []
                if len(waits) > 1:
                    for w in waits[:-1]:
                        out.append(
                            {
                                "engine": inst.get("engine"),
                                "ins": [],
                                "name": f"wsplit-{ctr}",
                                "opcode": "NoOp",
                                "outs": [],
                                "sync_info": {"on_update": [], "on_wait": [w]},
                            }
                        )
                        ctr += 1
                    si = dict(si)
                    si["on_wait"] = [waits[-1]]
                    inst = dict(inst)
                    inst["sync_info"] = si
                out.append(inst)
            blk["instructions"] = out
    return json.dumps(bir).encode()


_wait_patch_done = False


def _install_wait_legalizer():
    global _wait_patch_done
    if _wait_patch_done:
        return
    import concourse.bass_utils as bass_utils
    import concourse.bass2jax as bass2jax

    orig = bass_utils.compile_bir_kernel

    def wrapped(bir_json, tmpdir, neff_name="file.neff"):
        return orig(_legalize_waits_json(bir_json), tmpdir, neff_name)

    bass_utils.compile_bir_kernel = wrapped
    bass2jax.compile_bir_kernel = wrapped
    _wait_patch_done = True


def _build_nc():
    import concourse.bass as bass
    import concourse.mybir as mybir
    import concourse.tile as tile
    from concourse.alu_op_type import AluOpType

    f32 = mybir.dt.float32
    # float32r: same 4-byte storage as float32, but the PE processes the
    # moving operand at 1 cycle/row (vs 4 for float32) when N >= 256.
    f32r = mybir.dt.float32r
    nc = bass.Bass()
    feat = nc.dram_tensor("feat", [BPC, Cin, T, V], f32r, kind="ExternalInput")
    # wmat[k, c, o]: o-th contraction vector (u1/u2), c-chunk k of 128
    wmat = nc.dram_tensor("wmat", [2, 128, 2], f32r, kind="ExternalInput")
    amat = nc.dram_tensor("amat", [V, V], f32, kind="ExternalInput")
    cmat = nc.dram_tensor("cmat", [1, 1], f32, kind="ExternalInput")
    out = nc.dram_tensor("out", [BPC, T, V, V], f32, kind="ExternalOutput")

    with ExitStack() as ctx:
        tc = ctx.enter_context(tile.TileContext(nc))
        singles = ctx.enter_context(tc.tile_pool(name="singles", bufs=1))
        fpool = ctx.enter_context(tc.tile_pool(name="fpool", bufs=3))
        pspool = ctx.enter_context(tc.tile_pool(name="pspool", bufs=2, space="PSUM"))
        spool = ctx.enter_context(tc.tile_pool(name="spool", bufs=2))
        work = ctx.enter_context(tc.tile_pool(name="work", bufs=4))
        opool = ctx.enter_context(tc.tile_pool(name="opool", bufs=4))

        w_t = singles.tile([128, 2, 2], f32r)
        nc.sync.dma_start(out=w_t, in_=wmat[:, :, :].rearrange("k p o -> p k o"))
        a_bc = singles.tile([128, V, V], f32)
        nc.sync.dma_start(out=a_bc, in_=amat[:, :].partition_broadcast(128))
        c0_t = singles.tile([128, 1], f32)
        nc.sync.dma_start(out=c0_t, in_=cmat[0, :].partition_broadcast(128))

        # Absorb the const-DMA waits on cheap ops so steady-state instructions
        # carry fewer sync waits (less NoOp splitting at compile).
        warm_ps = pspool.tile([2, 2], f32, tag="warm")
        nc.tensor.matmul(
            out=warm_ps, lhsT=w_t[:, 0, :], rhs=w_t[:, 0, :], start=True, stop=True
        )
        scratch_c = singles.tile([128, 1], f32)
        nc.vector.tensor_copy(out=scratch_c, in_=c0_t)
        scratch_a = singles.tile([128, V, V], f32)
        nc.vector.tensor_copy(out=scratch_a, in_=a_bc)

        for b in range(BPC):
            # feat[b] as [c_within_chunk(partitions), chunk, t*v]
            f_t = fpool.tile([128, 2, TV], f32r)
            nc.sync.dma_start(
                out=f_t,
                in_=feat[b, :, :, :].rearrange("(k p) t v -> p k (t v)", p=128),
            )
            # s[o, t*v] = sum_c u_o[c] * feat[c, t*v]: w chunk stationary
            # (2-column LDW), feat moving in 512-wide slices, accumulated
            # over the two c-chunks in PSUM, staged to s_sb in SBUF.
            s_sb = spool.tile([2, TV], f32)
            NPS = TV // 1536  # 3 psum tiles of [2, 1536] (3 banks each)
            for sl in range(NPS):
                ps = pspool.tile([2, 1536], f32, tag="ps")
                for s in range(3):
                    lo = s * 512
                    for k in range(2):
                        nc.tensor.matmul(
                            out=ps[:, lo : lo + 512],
                            lhsT=w_t[:, k, :],
                            rhs=f_t[:, k, sl * 1536 + lo : sl * 1536 + lo + 512],
                            start=(k == 0),
                            stop=(k == 1),
                        )
                nc.scalar.copy(out=s_sb[:, sl * 1536 : (sl + 1) * 1536], in_=ps)
            for tb in range(NTB):
                # SBUF->SBUF scatter: s_sb[o, (t v)] row -> [t, v] tile
                s1t = work.tile([128, V], f32)
                nc.gpsimd.dma_start(
                    out=s1t,
                    in_=s_sb[0:1, tb * PB * V : (tb + 1) * PB * V].rearrange(
                        "o (t v) -> o t v", v=V
                    ),
                )
                s2t = work.tile([128, V], f32)
                nc.gpsimd.dma_start(
                    out=s2t,
                    in_=s_sb[1:2, tb * PB * V : (tb + 1) * PB * V].rearrange(
                        "o (t v) -> o t v", v=V
                    ),
                )
                sc = work.tile([128, V, V], f32)
                s1b = bass.AP(
                    tensor=s1t.tensor,
                    offset=s1t.offset,
                    ap=[s1t.ap[0], [1, V], [0, V]],
                )
                s2b = bass.AP(
                    tensor=s2t.tensor,
                    offset=s2t.offset,
                    ap=[s2t.ap[0], [0, V], [1, V]],
                )
                # sc = (s1 + c0) + s2
                nc.vector.scalar_tensor_tensor(
                    out=sc,
                    in0=s1b,
                    scalar=c0_t[:, :],
                    in1=s2b,
                    op0=AluOpType.add,
                    op1=AluOpType.add,
                )
                # LeakyReLU(x) = max(x, 0.1*x)
                lr = work.tile([128, V, V], f32)
                nc.vector.scalar_tensor_tensor(
                    out=lr,
                    in0=sc,
                    scalar=0.1,
                    in1=sc,
                    op0=AluOpType.mult,
                    op1=AluOpType.max,
                )
                ex = work.tile([128, V, V], f32)
                nc.scalar.activation(
                    out=ex, in_=lr, func=mybir.ActivationFunctionType.Exp
                )
                exa = work.tile([128, V, V], f32)
                nc.vector.tensor_mul(out=exa, in0=ex, in1=a_bc)
                ssum = work.tile([128, V], f32)
                nc.vector.reduce_sum(out=ssum, in_=exa, axis=mybir.AxisListType.X)
                rec = work.tile([128, V], f32)
                nc.vector.reciprocal(out=rec, in_=ssum)
                att = opool.tile([128, V, V], f32)
                rbc = bass.AP(
                    tensor=rec.tensor,
                    offset=rec.offset,
                    ap=[rec.ap[0], [1, V], [0, V]],
                )
                nc.vector.tensor_mul(out=att, in0=exa, in1=rbc)
                nc.gpsimd.dma_start(out=out[b, tb * PB : (tb + 1) * PB], in_=att)
    return nc


def _prep_params(Wf, bf, Wa, ba):
    w1, w2 = Wa[:64].astype(np.float64), Wa[64:].astype(np.float64)
    Wf64, bf64 = Wf.astype(np.float64), bf.astype(np.float64)
    u1 = w1 @ Wf64
    u2 = w2 @ Wf64
    c0 = float(w1 @ bf64 + w2 @ bf64 + float(ba[0]))
    wmat = np.stack([u1, u2], axis=-1).reshape(2, 128, 2).astype(np.float32)
    cmat = np.full((1, 1), c0, dtype=np.float32)
    return wmat, cmat


def get_nc():
    global _cached_nc
    if _cached_nc is None:
        _cached_nc = _build_nc()
    return _cached_nc


def kernel(feat, A, Wf, bf, Wa, ba):
    _install_wait_legalizer()
    from concourse.bass_utils import run_bass_kernel_spmd

    feat = np.ascontiguousarray(np.asarray(feat, dtype=np.float32))
    A = np.ascontiguousarray(np.asarray(A, dtype=np.float32))
    wmat, cmat = _prep_params(
        np.asarray(Wf, np.float32),
        np.asarray(bf, np.float32),
        np.asarray(Wa, np.float32),
        np.asarray(ba, np.float32),
    )

    nc = get_nc()
    in_maps = [
        {
            "feat": feat[i * BPC : (i + 1) * BPC],
            "wmat": wmat,
            "amat": A,
            "cmat": cmat,
        }
        for i in range(NCORES)
    ]
    res = run_bass_kernel_spmd(nc, in_maps, core_ids=list(range(NCORES)))
    return np.concatenate([r["out"] for r in res.results], axis=0)
